# revision 29
# baseline (speedup 1.0000x reference)
"""Trainium2 Bass kernel for nn_Mlp_cnn_shift (dense CNN MLP with 3x3 patch-shift
and a softmax-gated mix of two branches).

Strategy
--------
Data-parallel over the 16 (B,T) frames: each of the 8 NeuronCores processes 2
frames end-to-end.  All activations are kept channel-major ([C, tokens]) so the
channel contraction of every matmul has K on partitions, and `x` is
pre-transposed/cast on the host so no on-device transpose is needed.

Patch-shift handling:
 * forward shift (on xh, HID=1024): xh is stored in a zero-padded token layout
   (row pitch 57 = 56 cols + 1 zero pad col, 58-token zero guards per frame)
   and in 9 channel groups of 114 padded to 128 partitions each (host-permuted
   fc_w columns / fc1_w+fc2_w rows).  Every (dh,dw) roll then becomes a pure
   token offset in the fc1 matmul's rhs access pattern.
 * inverse shift (on gelu(y), C=512): y's channels are produced in 9 groups of
   57 padded to 64 partitions; the gelu PSUM evacuation writes each group
   directly into h at its inversely-shifted, edge-clipped token positions.

fc1 (h branch) and fc2 (w branch) run merged per row-group.  Their two M=64
half-blocks (shift group 8) are column-tiled into complementary 64-wide PE
array strips (h -> psum[64:128), w -> psum[0:64)) so the pair costs one matmul
slot instead of two.  The resulting split layout (h-g8 on partitions [64:120)
of block 4, w-g8 on [0:56)) is reconciled for free by the w DRAM spill/
readback DMA, and the gate-input row mapping is absorbed in host-side rw1
weight duplication.

proj bias is folded into the matmul via an all-ones row (q=512) of the gated
tensor whose gate value is pinned to 1 through +-40 logit biases; the gate's
softmax-of-2 is computed as sigmoid(l0-l1) and its gelu via Erf so the scalar
engine needs a single table set switch, prefetched off the critical path.
d = h - w is precomputed inside the frame loop (lagged one row-group), so
output gating is a single scalar_tensor_tensor per block.

The only cross-core coupling is the global (T,H,W) mean feeding the gate:
two tiny AllReduces, the first hidden under frame-1 compute.  TensorE is kept
warm across the second AllReduce with throwaway matmuls.

bf16 matmuls with f32 PSUM accumulation; output f32.  w spills to DRAM (bf16)
and streams back during the output phase.
"""

import os
import sys

for _p in ("/opt/trn_rl_repo",):
    if os.path.isdir(_p) and _p not in sys.path:
        sys.path.append(_p)

import numpy as np
import ml_dtypes

import concourse.bass as bass  # noqa: F401
import concourse.mybir as mybir
import concourse.tile as tile
from concourse import bacc
from concourse.bass_utils import run_bass_kernel_spmd

# ---------------------------------------------------------------- constants
SHIFTS = [(1, 1), (1, 0), (1, -1), (0, 1), (0, 0), (0, -1), (-1, 1), (-1, 0), (-1, -1)]
NG = 9
B, T, H, W, C = 2, 8, 56, 56, 512
HID = 1024
NCORES = 8
NF = (B * T) // NCORES          # frames per core = 2
HWTOK = H * W                   # 3136 tokens per frame
RP = W + 1                      # padded row pitch = 57
GUARD = RP + 1                  # 58 zero tokens on each end
FRPAD = RP * H                  # 3192
XHSPAN = GUARD + FRPAD + GUARD  # 3308
RG = 7                          # row groups per frame
RGR = H // RG                   # 8 rows per group
RGT = RGR * W                   # 448 valid tokens per row group
RGP = RGR * RP                  # 456 padded tokens per row group
GS_HID = 114                    # hid shift-group size (9*114 = 1026 >= 1024)
GS_C = 57                       # C shift-group size (9*57 = 513 >= 512)
GPAD = 64                       # C shift groups padded to 64 partitions
CP = NG * GPAD                  # 576 padded C rows
YCB = (CP + 127) // 128         # 5 row-blocks (last half-used)
CCB = C // 128                  # 4
HCB = HID // 128                # 8
NQ = YCB * 128                  # 640 padded q rows
G8N = C - GS_C * 8              # 56 channels in shift group 8
MEAN_N = float(T * H * W)
WARM_N = 32                     # head PE pre-warm junk matmuls

F32 = mybir.dt.float32
BF16 = mybir.dt.bfloat16
BF16_NP = ml_dtypes.bfloat16

_CACHE = {}


# ---------------------------------------------------------------- device kernel
def build_nc():
    nc = bacc.Bacc("TRN2", target_bir_lowering=False, debug=False, num_devices=NCORES)

    dp = nc.declare_dram_parameter
    xT = dp("xT", [NF, 128, CCB, HWTOK], BF16, isOutput=False)
    fcw = dp("fcw", [128, CCB, NG * 128], BF16, isOutput=False)
    fcb = dp("fcb", [128, NG], F32, isOutput=False)
    fc1w = dp("fc1w", [128, NG, CP], BF16, isOutput=False)
    fc1b = dp("fc1b", [128, YCB], F32, isOutput=False)
    fc2w = dp("fc2w", [128, NG, CP], BF16, isOutput=False)
    fc2b = dp("fc2b", [128, YCB], F32, isOutput=False)
    projw = dp("projw", [128, YCB, C], BF16, isOutput=False)
    projb = dp("projb", [128, C], F32, isOutput=False)
    rw1w = dp("rw1w", [128, YCB, 128], BF16, isOutput=False)
    rw1b = dp("rw1b", [128, 1], F32, isOutput=False)
    rw2w = dp("rw2w", [128, 2 * NQ], BF16, isOutput=False)
    rw2b = dp("rw2b", [128, 2 * YCB], F32, isOutput=False)
    bmask = dp("bmask", [128, B], F32, isOutput=False)
    out_d = dp("out", [NF, HWTOK, C], BF16, isOutput=True)

    # spill space for the w branch of each frame + collective bounce buffers
    wsp = [nc.dram_tensor(f"wsp{f}", [128, YCB, HWTOK], BF16) for f in range(NF)]
    ccin = [nc.dram_tensor(f"ccin{f}", [B, 128, YCB], F32) for f in range(NF)]
    ccout = [
        nc.dram_tensor(f"ccout{f}", [B, 128, YCB], F32, addr_space="Shared")
        for f in range(NF)
    ]

    AF = mybir.ActivationFunctionType
    ALU = mybir.AluOpType

    with tile.TileContext(nc, num_cores=NCORES) as tc:
        with (
            tc.tile_pool(name="singles", bufs=1) as singles,
            tc.tile_pool(name="xh_pool", bufs=1) as xh_pool,
            tc.tile_pool(name="h_pool", bufs=2) as h_pool,
            tc.tile_pool(name="w_pool", bufs=2) as w_pool,
            tc.tile_pool(name="xt_pool", bufs=3) as xt_pool,
            tc.tile_pool(name="w4_pool", bufs=2) as w4_pool,
            tc.tile_pool(name="ostage", bufs=3) as ostage,
            tc.tile_pool(name="dstream", bufs=3) as dstream,
            tc.tile_pool(name="small", bufs=1) as small,
            tc.tile_pool(name="mmpsum", bufs=8, space="PSUM") as mmpsum,
        ):
            # ---- head: PE pre-warm + gelu table prefetch while DMAs land
            junkbuf = singles.tile([128, 128], BF16, name="junkbuf")
            nc.vector.memset(junkbuf[:], 0.0)
            psJ = mmpsum.tile([128, 512], F32, tag="mm", name="warmhead")
            for _ in range(WARM_N):
                nc.tensor.matmul(
                    psJ[0:64, 0:128],
                    lhsT=junkbuf[:, 0:64],
                    rhs=junkbuf[:, 0:128],
                    start=True,
                    stop=True,
                )
            gel0 = singles.tile([128, 1], BF16, name="gel0")
            nc.scalar.activation(out=gel0, in_=junkbuf[:, 0:1], func=AF.Gelu)

            # first fc weight chunk + x tile, then the rest, so mb0/1 can
            # start as early as possible
            fcw_s = singles.tile([128, CCB, NG * 128], BF16, name="fcw_s")
            nc.sync.dma_start(out=fcw_s[:, :, 0:256], in_=fcw[:, :, 0:256])
            xt00 = xt_pool.tile([128, CCB, RGT], BF16, tag="xt")
            nc.sync.dma_start(out=xt00, in_=xT[0, :, :, 0:RGT])
            fcb_s = singles.tile([128, NG], F32, name="fcb_s")
            nc.sync.dma_start(out=fcb_s, in_=fcb[:])
            nc.sync.dma_start(out=fcw_s[:, :, 256:], in_=fcw[:, :, 256:])

            def load(name, shape, dtype, src):
                t = singles.tile(shape, dtype, name=name)
                nc.sync.dma_start(out=t, in_=src[:])
                return t

            _rest = {}

            def load_rest():
                _rest["fc1w_s"] = load("fc1w_s", [128, NG, CP], BF16, fc1w)
                _rest["fc1b_s"] = load("fc1b_s", [128, YCB], F32, fc1b)
                _rest["fc2w_s"] = load("fc2w_s", [128, NG, CP], BF16, fc2w)
                _rest["fc2b_s"] = load("fc2b_s", [128, YCB], F32, fc2b)
                _rest["projw_s"] = load("projw_s", [128, YCB, C], BF16, projw)
                _rest["projb_s"] = load("projb_s", [128, C], F32, projb)
                _rest["rw1w_s"] = load("rw1w_s", [128, YCB, 128], BF16, rw1w)
                _rest["rw1b_s"] = load("rw1b_s", [128, 1], F32, rw1b)
                _rest["rw2w_s"] = load("rw2w_s", [128, 2 * NQ], BF16, rw2w)
                _rest["rw2b_s"] = load("rw2b_s", [128, 2 * YCB], F32, rw2b)
                _rest["bmask_s"] = load("bmask_s", [128, B], F32, bmask)

            a0_s = singles.tile([128, YCB], F32)   # gate for the h branch

            # xh, padded token layout, persistent across frames.
            xh = xh_pool.tile([128, NG, XHSPAN], BF16)
            nc.vector.memset(xh[:, :, :GUARD], 0.0)
            nc.vector.memset(xh[:, :, GUARD + FRPAD:], 0.0)
            xh_rows = xh[:, :, GUARD:GUARD + FRPAD].rearrange(
                "p g (r c) -> p g r c", c=RP
            )
            nc.vector.memset(xh_rows[:, :, :, W:], 0.0)

            hw_tiles = []
            part_sums = []

            for f in range(NF):
                # ---------------- A: xh = gelu(x @ fc_w + fc_b), group-blocked
                for rg in range(RG):
                    if f == 0 and rg == 0:
                        xt_t = xt00
                    else:
                        xt_t = xt_pool.tile([128, CCB, RGT], BF16, tag="xt")
                        nc.sync.dma_start(
                            out=xt_t, in_=xT[f, :, :, rg * RGT:(rg + 1) * RGT]
                        )
                    for mb in range(NG):
                        ps = mmpsum.tile([128, 512], F32, tag="mm")
                        for k in range(CCB):
                            nc.tensor.matmul(
                                ps[:, :RGT],
                                lhsT=fcw_s[:, k, mb * 128:(mb + 1) * 128],
                                rhs=xt_t[:, k, :],
                                start=(k == 0),
                                stop=(k == CCB - 1),
                            )
                        dst = xh[
                            :, mb, GUARD + rg * RGP:GUARD + (rg + 1) * RGP
                        ].rearrange("p (r c) -> p r c", c=RP)[:, :, :W]
                        src = ps[:, :RGT].rearrange("p (r c) -> p r c", c=W)
                        nc.scalar.activation(
                            out=dst, in_=src, func=AF.Gelu,
                            bias=fcb_s[:, mb:mb + 1],
                        )

                if f == 0:
                    # frame-0 fc pass is in flight; now bring in the rest
                    load_rest()
                    fc1w_s = _rest["fc1w_s"]; fc1b_s = _rest["fc1b_s"]
                    fc2w_s = _rest["fc2w_s"]; fc2b_s = _rest["fc2b_s"]
                    projw_s = _rest["projw_s"]; projb_s = _rest["projb_s"]
                    rw1w_s = _rest["rw1w_s"]; rw1b_s = _rest["rw1b_s"]
                    rw2w_s = _rest["rw2w_s"]; rw2b_s = _rest["rw2b_s"]
                    bmask_s = _rest["bmask_s"]

                # ---------------- C+B merged: h and w per row-group
                h_t = h_pool.tile([128, YCB, HWTOK], BF16, tag="h")
                nc.gpsimd.memset(h_t[:], 0.0)
                # zero-fill the never-spilled kb4 lower half of wsp from the
                # freshly zeroed h tile, then set the all-ones bias row (q=512)
                nc.sync.dma_start(out=wsp[f][0:64, 4, :], in_=h_t[0:64, 4, :])
                nc.vector.memset(h_t[0:1, 4, :], 1.0)
                h4 = h_t.rearrange("p c (i j) -> p c i j", j=W)
                hsum_st = small.tile([128, YCB, RG], F32, tag=f"hsst{f}")
                nc.vector.memset(hsum_st[:], 0.0)
                wsum_st = small.tile([128, YCB, RG], F32, tag=f"wsst{f}")
                nc.vector.memset(wsum_st[:], 0.0)

                def emit_d(h_tt, w_tt, rgd):
                    t0 = rgd * RGT
                    for kb in range(4):
                        nc.vector.tensor_tensor(
                            h_tt[:, kb, t0:t0 + RGT],
                            h_tt[:, kb, t0:t0 + RGT],
                            w_tt[:, kb, :],
                            ALU.subtract,
                        )

                w_tiles = []
                for rg in range(RG):
                    w_rg_t = w_pool.tile([128, YCB, RGT], BF16, tag="wrg")
                    s0p = GUARD + rg * RGP
                    for mb in range(4):
                        # fc1 chain (h), full 128-wide M block
                        psH = mmpsum.tile([128, 512], F32, tag="mm")
                        for g in range(NG):
                            off = -(SHIFTS[g][0] * RP + SHIFTS[g][1])
                            rhs2 = xh[:, g, s0p + off:s0p + off + RGP].rearrange(
                                "p (r c) -> p r c", c=RP
                            )[:, :, :W]
                            nc.tensor.matmul(
                                psH[:, :RGT],
                                lhsT=fc1w_s[:, g, mb * 128:(mb + 1) * 128],
                                rhs=rhs2,
                                start=(g == 0),
                                stop=(g == NG - 1),
                            )
                        ps3 = psH[:, :RGT].rearrange("p (r c) -> p r c", c=W)
                        for half in range(2):
                            g = 2 * mb + half
                            nch = min(GS_C * (g + 1), C) - GS_C * g
                            sh, sw = SHIFTS[g]
                            i0 = max(0, 8 * rg - sh)
                            i1 = min(H, 8 * rg + 8 - sh)
                            j0, j1 = max(0, -sw), min(W, W - sw)
                            p0 = half * GPAD
                            nc.scalar.activation(
                                out=h4[p0:p0 + nch, mb, i0:i1, j0:j1],
                                in_=ps3[
                                    p0:p0 + nch,
                                    i0 + sh - 8 * rg:i1 + sh - 8 * rg,
                                    j0 + sw:j1 + sw,
                                ],
                                func=AF.Gelu,
                                bias=fc1b_s[p0:p0 + nch, mb:mb + 1],
                                accum_out=hsum_st[p0:p0 + nch, mb, rg:rg + 1],
                            )
                        # fc2 chain (w), full 128-wide M block
                        psW = mmpsum.tile([128, 512], F32, tag="mm")
                        for g in range(NG):
                            rhs2 = xh[:, g, s0p:s0p + RGP].rearrange(
                                "p (r c) -> p r c", c=RP
                            )[:, :, :W]
                            nc.tensor.matmul(
                                psW[:, :RGT],
                                lhsT=fc2w_s[:, g, mb * 128:(mb + 1) * 128],
                                rhs=rhs2,
                                start=(g == 0),
                                stop=(g == NG - 1),
                            )
                        nc.scalar.activation(
                            out=w_rg_t[:, mb, :], in_=psW[:, :RGT], func=AF.Gelu,
                            bias=fc2b_s[:, mb:mb + 1],
                            accum_out=wsum_st[:, mb, rg:rg + 1],
                        )

                    # mb4: the two M=64 half blocks (shift group 8) of fc1 and
                    # fc2, column-tiled into complementary 64-wide PE strips so
                    # the pair streams concurrently (one matmul slot each g).
                    psH4 = mmpsum.tile([128, 512], F32, tag="mm")
                    psW4 = mmpsum.tile([128, 512], F32, tag="mm")
                    for g in range(NG):
                        off = -(SHIFTS[g][0] * RP + SHIFTS[g][1])
                        rhsS = xh[:, g, s0p + off:s0p + off + RGP].rearrange(
                            "p (r c) -> p r c", c=RP
                        )[:, :, :W]
                        rhsP = xh[:, g, s0p:s0p + RGP].rearrange(
                            "p (r c) -> p r c", c=RP
                        )[:, :, :W]
                        nc.tensor.matmul(
                            psH4[64:128, :RGT],
                            lhsT=fc1w_s[:, g, 512:576],
                            rhs=rhsS,
                            start=(g == 0),
                            stop=(g == NG - 1),
                        )
                        nc.tensor.matmul(
                            psW4[0:64, :RGT],
                            lhsT=fc2w_s[:, g, 512:576],
                            rhs=rhsP,
                            start=(g == 0),
                            stop=(g == NG - 1),
                        )
                    ps3H4 = psH4[:, :RGT].rearrange("p (r c) -> p r c", c=W)
                    sh, sw = SHIFTS[8]
                    i0 = max(0, 8 * rg - sh)
                    i1 = min(H, 8 * rg + 8 - sh)
                    j0, j1 = max(0, -sw), min(W, W - sw)
                    nc.scalar.activation(
                        out=h4[64:64 + G8N, 4, i0:i1, j0:j1],
                        in_=ps3H4[
                            64:64 + G8N,
                            i0 + sh - 8 * rg:i1 + sh - 8 * rg,
                            j0 + sw:j1 + sw,
                        ],
                        func=AF.Gelu,
                        bias=fc1b_s[64:64 + G8N, 4:5],
                        accum_out=hsum_st[64:64 + G8N, 4, rg:rg + 1],
                    )
                    # full 64-lane strip: lanes [G8N:64) carry gelu(0)=0 from
                    # the zero-padded weight cols, keeping the spill NaN-free
                    nc.scalar.activation(
                        out=w_rg_t[0:64, 4, :], in_=psW4[0:64, :RGT],
                        func=AF.Gelu,
                        bias=fc2b_s[0:64, 4:5],
                        accum_out=wsum_st[0:64, 4, rg:rg + 1],
                    )

                    # spill w: kb0-3 straight, kb4's w-g8 remapped up to the
                    # h-aligned partitions [64:120) via the DRAM roundtrip
                    nc.sync.dma_start(
                        out=wsp[f][:, 0:4, rg * RGT:(rg + 1) * RGT],
                        in_=w_rg_t[:, 0:4, :],
                    )
                    nc.sync.dma_start(
                        out=wsp[f][64:128, 4, rg * RGT:(rg + 1) * RGT],
                        in_=w_rg_t[0:64, 4, :],
                    )
                    w_tiles.append(w_rg_t)
                    # d for the g8 half-block: remap w-g8 up to the h-aligned
                    # partitions with a tiny SBUF DMA, then subtract in place
                    w4r = w4_pool.tile([128, RGT], BF16, tag="w4r")
                    nc.sync.dma_start(out=w4r[64:128, :], in_=w_rg_t[0:64, 4, :])
                    nc.vector.tensor_tensor(
                        h_t[64:128, 4, rg * RGT:(rg + 1) * RGT],
                        h_t[64:128, 4, rg * RGT:(rg + 1) * RGT],
                        w4r[64:128, :],
                        ALU.subtract,
                    )
                    # d = h - w for the previous row group (its h rows are
                    # final once this pass's evacuations have run)
                    if rg >= 1:
                        emit_d(h_t, w_tiles[rg - 1], rg - 1)
                emit_d(h_t, w_tiles[RG - 1], RG - 1)

                # ---------------- per-frame gate partial sum + AllReduce
                hs = small.tile([128, YCB], F32, tag=f"hs{f}")
                nc.vector.tensor_reduce(
                    out=hs, in_=hsum_st[:], axis=mybir.AxisListType.X, op=ALU.add
                )
                ws = small.tile([128, YCB], F32, tag=f"ws{f}")
                nc.vector.tensor_reduce(
                    out=ws, in_=wsum_st[:], axis=mybir.AxisListType.X, op=ALU.add
                )
                part = small.tile([128, YCB], F32, tag=f"part{f}")
                nc.vector.tensor_tensor(part, hs, ws, ALU.add)
                part_sums.append(part)
                t0 = small.tile([128, YCB], F32, tag=f"cca{f}")
                nc.vector.tensor_scalar_mul(t0, part, bmask_s[:, 0:1])
                t1 = small.tile([128, YCB], F32, tag=f"ccb{f}")
                nc.vector.tensor_scalar_mul(t1, part, bmask_s[:, 1:2])
                nc.sync.dma_start(out=ccin[f][0], in_=t0)
                nc.sync.dma_start(out=ccin[f][1], in_=t1)
                nc.gpsimd.collective_compute(
                    "AllReduce",
                    ALU.add,
                    replica_groups=[list(range(NCORES))],
                    ins=[ccin[f][:]],
                    outs=[ccout[f][:]],
                )

                hw_tiles.append(h_t)

            # keep TensorE's activity monitor warm across the second
            # AllReduce's latency window (junk matmuls, results unread).
            # Decaying N: coverage when the collective is slow, fast FIFO
            # drain when it is quick.
            for wi, wn in enumerate([512] * 70 + [256] * 50 + [128] * 50):
                wp = mmpsum.tile([128, 512], F32, tag="mm", name=f"warm{wi}")
                nc.tensor.matmul(
                    wp[:, :wn],
                    lhsT=fcw_s[:, 0, 0:128],
                    rhs=fcw_s[:, 1, 0:wn],
                    start=True,
                    stop=True,
                )
            # prefetch the erf/sigmoid activation table set while waiting
            dum0 = small.tile([128, 1], F32, tag="dum0")
            nc.scalar.activation(out=dum0, in_=junkbuf[:, 0:1], func=AF.Erf)
            # tiny keep-warm matmuls spanning the post-collective gate
            # pipeline (DMA-in + gate math) so the output phase starts warm
            for wi in range(40):
                wp = mmpsum.tile([128, 512], F32, tag="mm", name=f"gwarm{wi}")
                nc.tensor.matmul(
                    wp[:, :96],
                    lhsT=fcw_s[:, 0, 0:128],
                    rhs=fcw_s[:, 1, 0:96],
                    start=True,
                    stop=True,
                )

            # ---------------- combine the two AllReduce results -> z
            # zsum = (za0+za1)*m0 + (zb0+zb1)*m1: 4 DVE ops + cast
            zt4 = []
            for f in range(NF):
                za = small.tile([128, YCB], F32, tag=f"za{f}")
                nc.sync.dma_start(out=za, in_=ccout[f][0])
                zb = small.tile([128, YCB], F32, tag=f"zb{f}")
                nc.sync.dma_start(out=zb, in_=ccout[f][1])
                zt4.append((za, zb))
            s_a = small.tile([128, YCB], F32, tag="s_a")
            nc.vector.tensor_tensor(s_a, zt4[0][0], zt4[1][0], ALU.add)
            s_b = small.tile([128, YCB], F32, tag="s_b")
            nc.vector.tensor_tensor(s_b, zt4[0][1], zt4[1][1], ALU.add)
            nc.vector.tensor_scalar_mul(s_b, s_b, bmask_s[:, 1:2])
            zsum = small.tile([128, YCB], F32, tag="zsum")
            nc.vector.scalar_tensor_tensor(
                zsum, s_a, bmask_s[:, 0:1], s_b, ALU.mult, ALU.add
            )
            zbf = small.tile([128, YCB], BF16, tag="zbf")
            nc.vector.tensor_copy(out=zbf, in_=zsum)

            # ---------------- gate: a0 = sigmoid(l0 - l1)
            # (1/MEAN_N folded into rw1w; 0.5 of the exact erf-gelu folded
            # into rw2w, both host-side)
            psg = mmpsum.tile([128, 512], F32, tag="mm", name="psg")[:, :1]
            for k in range(YCB):
                nc.tensor.matmul(
                    psg,
                    lhsT=rw1w_s[:, k, :],
                    rhs=zbf[:, k:k + 1],
                    start=(k == 0),
                    stop=(k == YCB - 1),
                )
            zt = small.tile([128, 1], F32, tag="zt")
            nc.vector.tensor_scalar_add(zt, psg, rw1b_s[:, 0:1])
            ert = small.tile([128, 1], F32, tag="ert")
            nc.scalar.activation(
                out=ert, in_=zt, func=AF.Erf, scale=0.7071067811865476
            )
            gv = small.tile([128, 1], BF16, tag="gv")
            nc.vector.scalar_tensor_tensor(gv, ert, 1.0, zt, ALU.add, ALU.mult)
            psu = mmpsum.tile([128, 512], F32, tag="mm", name="psu")[:, :2 * YCB]
            for m in range(2 * YCB):
                nc.tensor.matmul(
                    psu[:, m:m + 1],
                    lhsT=rw2w_s[:, m * 128:(m + 1) * 128],
                    rhs=gv,
                    start=True,
                    stop=True,
                )
            uv = small.tile([128, 2 * YCB], F32, tag="uv")
            nc.vector.tensor_tensor(uv, psu, rw2b_s, ALU.add)
            sg = small.tile([128, YCB], F32, tag="sg")
            nc.vector.tensor_tensor(
                sg, uv[:, 0:YCB], uv[:, YCB:2 * YCB], ALU.subtract
            )
            nc.scalar.activation(out=a0_s, in_=sg, func=AF.Sigmoid)

            # ---------------- D: out = (w + a0*d) @ proj_w   (bias in-matmul)
            def proj_blocks(gated_ap, fidx, tbase, ntok):
                m0 = 0
                blk = 0
                while m0 < ntok:
                    M = min(128, ntok - m0)
                    pp = mmpsum.tile([128, 512], F32, tag="mm")
                    for kb in range(YCB):
                        nc.tensor.matmul(
                            pp[:M, :C],
                            lhsT=gated_ap[:, kb, m0:m0 + M],
                            rhs=projw_s[:, kb, :],
                            start=(kb == 0),
                            stop=(kb == YCB - 1),
                        )
                    ot = ostage.tile([128, C], BF16, tag="ot")
                    nc.scalar.copy(ot[:M], pp[:M, :C])
                    nc.sync.dma_start(
                        out=out_d[fidx, tbase + m0:tbase + m0 + M, :], in_=ot[:M]
                    )
                    m0 += M
                    blk += 1

            chunks_by_frame = {
                1: [128, 512, 512, 512, 512, 512, 448],
                0: [512, 512, 512, 512, 512, 512, 64],
            }
            for fidx in (1, 0):
                h_t = hw_tiles[fidx]
                offs = []
                ck0 = 0
                for CK in chunks_by_frame[fidx]:
                    offs.append((ck0, CK))
                    ck0 += CK
                # w readback prefetched two chunks ahead: in program order each
                # wc DMA then precedes the output writes of the chunk two back,
                # so it never waits behind them in the queue
                wcs = {}

                def issue_wc(j, fi=fidx, of=offs, wl=wcs):
                    c0, CKj = of[j]
                    t = dstream.tile([128, YCB, 512], BF16, tag="wc")
                    nc.sync.dma_start(
                        out=t[:, :, :CKj], in_=wsp[fi][:, :, c0:c0 + CKj]
                    )
                    wl[j] = t

                issue_wc(0)
                issue_wc(1)
                for j, (ck0, CK) in enumerate(offs):
                    if j + 2 < len(offs):
                        issue_wc(j + 2)
                    wc = wcs.pop(j)
                    # gated = a0*d + w, in place over d
                    for kb in range(YCB):
                        eng = nc.vector
                        eng.scalar_tensor_tensor(
                            h_t[:, kb, ck0:ck0 + CK],
                            h_t[:, kb, ck0:ck0 + CK],
                            a0_s[:, kb:kb + 1],
                            wc[:, kb, :CK],
                            ALU.mult,
                            ALU.add,
                        )
                    proj_blocks(h_t[:, :, ck0:ck0 + CK], fidx, ck0, CK)

    nc.compile()
    return nc


# ---------------------------------------------------------------- host side
def _prep_weights(fc_w, fc_b, fc1_w, fc1_b, fc2_w, fc2_b,
                  rw1_w, rw1_b, rw2_w, rw2_b, proj_w, proj_b):
    f32 = np.float32

    # weight-column layout (fc1/fc2 outputs): group g at cols [64g, 64g+nch)
    qof_col = np.full((CP,), -1, np.int64)
    for g in range(NG):
        c0 = GS_C * g
        c1 = min(GS_C * (g + 1), C)
        qof_col[GPAD * g:GPAD * g + (c1 - c0)] = np.arange(c0, c1)
    qv_col = qof_col >= 0
    qi_col = np.where(qv_col, np.maximum(qof_col, 0), 0)

    # canonical q-row layout of h/gated (640 rows): groups 0-7 at [64g,64g+57),
    # group 8 at [576, 576+56) (the column-tiled upper strip); q=512 is the
    # all-ones bias row
    qof_h = np.full((NQ,), -1, np.int64)
    for g in range(8):
        qof_h[GPAD * g:GPAD * g + GS_C] = np.arange(GS_C * g, GS_C * (g + 1))
    qof_h[576:576 + G8N] = np.arange(GS_C * 8, C)
    qv_h = qof_h >= 0
    qi_h = np.where(qv_h, np.maximum(qof_h, 0), 0)

    def cols_to_padded576(m):  # [R, C] -> [R, CP] with zero pad cols
        out = np.zeros((m.shape[0], CP), f32)
        out[:, qv_col] = m[:, qi_col[qv_col]]
        return out

    # fc: columns permuted into 9 HID-groups of 114 (112 for g=8), pad to 128
    fcwp = np.zeros((C, NG * 128), f32)
    fcbp = np.zeros((NG * 128,), f32)
    for g in range(NG):
        n = min(GS_HID * (g + 1), HID) - GS_HID * g
        fcwp[:, 128 * g:128 * g + n] = fc_w[:, GS_HID * g:GS_HID * g + n]
        fcbp[128 * g:128 * g + n] = fc_b[GS_HID * g:GS_HID * g + n]
    fcw_h = np.ascontiguousarray(
        fcwp.reshape(CCB, 128, NG * 128).transpose(1, 0, 2)
    ).astype(BF16_NP)
    fcb_h = np.ascontiguousarray(fcbp.reshape(NG, 128).T).astype(f32)

    def hid_rows_grouped(wm):  # [HID, CP] -> [128, NG, CP] padded group rows
        wp = np.zeros((NG * 128, wm.shape[1]), f32)
        for g in range(NG):
            n = min(GS_HID * (g + 1), HID) - GS_HID * g
            wp[128 * g:128 * g + n] = wm[GS_HID * g:GS_HID * g + n]
        return np.ascontiguousarray(
            wp.reshape(NG, 128, wm.shape[1]).transpose(1, 0, 2)
        ).astype(BF16_NP)

    fc1w_h = hid_rows_grouped(cols_to_padded576(fc1_w))
    fc2w_h = hid_rows_grouped(cols_to_padded576(fc2_w))

    # fc1 bias follows the h q-layout (g8 on lanes [64:120) of block 4);
    # fc2 bias follows the column layout (g8 on lanes [0:56) of block 4)
    fc1b_full = np.zeros((NQ,), f32)
    fc1b_full[qv_h] = fc1_b[qi_h[qv_h]]
    fc1b_h = np.ascontiguousarray(fc1b_full.reshape(YCB, 128).T).astype(f32)
    fc2b_full = np.zeros((NQ,), f32)
    fc2b_full[:CP][qv_col] = fc2_b[qi_col[qv_col]]
    fc2b_h = np.ascontiguousarray(fc2b_full.reshape(YCB, 128).T).astype(f32)

    # proj: rows in h q-layout; row 512 carries the bias (ones row of gated)
    projwp = np.zeros((NQ, C), f32)
    projwp[qv_h] = proj_w[qi_h[qv_h]]
    projwp[512] = proj_b
    projw_h = np.ascontiguousarray(
        projwp.reshape(YCB, 128, C).transpose(1, 0, 2)
    ).astype(BF16_NP)
    projb_h = np.ascontiguousarray(
        np.broadcast_to(proj_b[None, :], (128, C))
    ).astype(f32)

    # rw1 rows, scaled by 1/MEAN_N: h-sums land on the h q-rows, w-sums for
    # group 8 land on lanes [512, 512+56) (the column-layout block-4 lower
    # strip) -- duplicate the g8 rows there so psg sums h+w contributions
    rw1p = np.zeros((NQ, C // 4), f32)
    rw1p[qv_h] = rw1_w[qi_h[qv_h]] / MEAN_N
    rw1p[512:512 + G8N] = rw1_w[GS_C * 8:C] / MEAN_N
    rw1w_h = np.ascontiguousarray(
        rw1p.reshape(YCB, 128, C // 4).transpose(1, 0, 2)
    ).astype(BF16_NP)
    rw1b_h = np.ascontiguousarray(rw1_b[:, None]).astype(f32)

    # rw2 columns in the h q-layout (x0.5: exact-gelu via erf gives
    # gv = 2*gelu(z)); stream-0 logits at [0, NQ), stream-1 at [NQ, 2NQ).
    # Pad rows get -40/+40 logit biases (gate -> 0); the ones row q=512
    # gets +40/-40 (gate -> 1) so the proj bias row passes through.
    rw2p = np.zeros((128, 2 * NQ), f32)
    rw2p[:, 0:NQ][:, qv_h] = 0.5 * rw2_w[:, 2 * qi_h[qv_h]]
    rw2p[:, NQ:2 * NQ][:, qv_h] = 0.5 * rw2_w[:, 2 * qi_h[qv_h] + 1]
    rw2w_h = np.ascontiguousarray(rw2p).astype(BF16_NP)
    rw2b_full = np.zeros((2 * NQ,), f32)
    rw2b_full[0:NQ][qv_h] = rw2_b[2 * qi_h[qv_h]]
    rw2b_full[NQ:2 * NQ][qv_h] = rw2_b[2 * qi_h[qv_h] + 1]
    rw2b_full[0:NQ][~qv_h] = -40.0
    rw2b_full[NQ:2 * NQ][~qv_h] = 40.0
    rw2b_full[512] = 40.0
    rw2b_full[NQ + 512] = -40.0
    rw2b_h = np.ascontiguousarray(rw2b_full.reshape(2 * YCB, 128).T).astype(f32)

    return dict(
        fcw=fcw_h, fcb=fcb_h, fc1w=fc1w_h, fc1b=fc1b_h, fc2w=fc2w_h,
        fc2b=fc2b_h, projw=projw_h, projb=projb_h, rw1w=rw1w_h, rw1b=rw1b_h,
        rw2w=rw2w_h, rw2b=rw2b_h,
    )


def _get_nc():
    if "nc" not in _CACHE:
        _CACHE["nc"] = build_nc()
    return _CACHE["nc"]


def run(inputs, trace=False, trace_kwargs=None):
    """Run the SPMD kernel; returns (full_output, BassKernelResults)."""
    x = np.asarray(inputs["x"], np.float32)
    shared = _prep_weights(
        np.asarray(inputs["fc_w"], np.float32), np.asarray(inputs["fc_b"], np.float32),
        np.asarray(inputs["fc1_w"], np.float32), np.asarray(inputs["fc1_b"], np.float32),
        np.asarray(inputs["fc2_w"], np.float32), np.asarray(inputs["fc2_b"], np.float32),
        np.asarray(inputs["rw1_w"], np.float32), np.asarray(inputs["rw1_b"], np.float32),
        np.asarray(inputs["rw2_w"], np.float32), np.asarray(inputs["rw2_b"], np.float32),
        np.asarray(inputs["proj_w"], np.float32), np.asarray(inputs["proj_b"], np.float32),
    )

    xf = x.reshape(B * T, HWTOK, C)
    in_maps = []
    for c in range(NCORES):
        sh = xf[NF * c:NF * (c + 1)]                      # [NF, 3136, 512]
        xt = sh.transpose(0, 2, 1).reshape(NF, CCB, 128, HWTOK)
        xt = np.ascontiguousarray(xt.transpose(0, 2, 1, 3)).astype(BF16_NP)
        bm = np.zeros((128, B), np.float32)
        bm[:, (NF * c) // T] = 1.0
        m = dict(shared)
        m["xT"] = xt
        m["bmask"] = bm
        in_maps.append(m)

    nc = _get_nc()
    res = run_bass_kernel_spmd(
        nc, in_maps, list(range(NCORES)),
        trace=trace, **(dict(trace_kwargs=trace_kwargs) if trace_kwargs else {}),
    )

    out = np.empty((B * T, HWTOK, C), np.float32)
    for c in range(NCORES):
        out[NF * c:NF * (c + 1)] = res.results[c]["out"].astype(np.float32)
    return out.reshape(B, T, H, W, C), res


def kernel(**inputs) -> np.ndarray:
    full, _ = run(inputs, trace=False)
    return full
